# revision 1
# baseline (speedup 1.0000x reference)
"""EnhancedGCN (GCNConv + GATConv) Trainium2 Bass kernel, 8-core SPMD.

Strategy (dst-node partitioned, per the sharding hint):
  - Host: sort edges by destination block, pad to uniform per-block tile
    counts across cores (one SPMD program), compute degrees (the CSR
    histogram) as part of partitioning. Self-loop terms are handled
    analytically per block on device (no gather needed for them).
  - Phase A (replicated): every core computes the full scaled table
    xw' = deg^-1/2 * (x @ W1)  ->  table1 [N, 128] in local HBM.
  - Phase B (dst-partitioned): per 128-dst block, per 128-edge tile:
    indirect-DMA gather of xw'[src] rows, one-hot selection matrix from
    dst-local ids (DVE is_equal vs iota), PE matmul accumulation into
    PSUM. Block epilogue: add self-loop term, scale by deg^-1/2[dst],
    +b1, ReLU -> h; transpose h; one matmul with [W2 | S'src | S'dst]
    gives [h2 | a_s | a_d] rows -> staged for exchange.
  - AllGather of the per-core [h2 | a_s | a_d] slices -> table2 [N, 200].
  - Phase C: per edge tile: gather table2[src] rows ([h2|a_s]), replicate
    a_d[dst] to edges via PE transpose + one-hot matmul, logits =
    a_s+a_d -> LeakyReLU -> Exp (no max-subtraction; logits are O(10)),
    scale h2 rows by per-head weights, matmul with one-hot accumulates
    [weighted h2 | weight] per dst; epilogue adds the self-loop term,
    divides by the summed weights, +b2, ReLU -> output rows.
"""
import math
import numpy as np

import concourse.bass as bass
from concourse import bacc
import concourse.mybir as mybir
import concourse.tile as tile
from concourse.masks import make_identity
from concourse.bass_utils import run_bass_kernel_spmd

P = 128
F32 = mybir.dt.float32
I32 = mybir.dt.int32
U8 = mybir.dt.uint8
AF = mybir.ActivationFunctionType
ALU = mybir.AluOpType

# Problem constants (tests may override N before calling kernel())
N = 50000
IN_CH = 256
HID = 128
HEADS = 3
C_OUT = 64
OUT = HEADS * C_OUT  # 192
NEG_SLOPE = 0.2
NCORES = 8
T2W = 200            # table2 row width (192 h2 + 3 a_s + 3 a_d + 2 pad)
IC = 512             # idx chunk columns
NW = OUT + HEADS     # 195: [G' | w] matmul width


def _sizes():
    npc = N // NCORES
    nblk = math.ceil(npc / P)
    lastb = npc - (nblk - 1) * P
    ncols = math.ceil(N / P)
    nrows = ncols * P + P  # extra zero rows for pad gathers
    return npc, nblk, lastb, ncols, nrows


def _bcast_heads(ap):
    """[P, HEADS] AP -> [P, HEADS, C_OUT] zero-step broadcast AP."""
    return bass.AP(ap.tensor, ap.offset, [ap.ap[0], [1, HEADS], [0, C_OUT]])


def _host_prep(x, edge_index, W1, b1, W2, att_src, att_dst, b2):
    npc, nblk, lastb, ncols, nrows = _sizes()
    src = np.asarray(edge_index[0], dtype=np.int64)
    dst = np.asarray(edge_index[1], dtype=np.int64)
    deg = np.bincount(dst, minlength=N).astype(np.float64) + 1.0  # + self loop

    c = dst // npc
    r = dst - c * npc
    b = r // P
    cb = (c * nblk + b).astype(np.int64)
    dloc = (r - b * P).astype(np.int64)

    nbuckets = NCORES * nblk
    counts = np.bincount(cb, minlength=nbuckets).reshape(NCORES, nblk)
    tb = np.maximum(1, np.ceil(counts.max(axis=0) / P).astype(np.int64))  # [nblk]
    TT = int(tb.sum())
    nch = math.ceil(TT / IC)
    icc = min(IC, TT)

    order = np.argsort(cb, kind="stable")
    src_s = src[order]
    dloc_s = dloc[order]
    cb_s = cb[order]
    starts = np.searchsorted(cb_s, np.arange(nbuckets))
    ends = np.searchsorted(cb_s, np.arange(nbuckets) + 1)

    idxsrc = np.zeros((NCORES, nch * icc, P), dtype=np.int32)
    dlarr = np.full((NCORES, nch * icc, P), 255, dtype=np.uint8)
    tile_ofs = np.concatenate([[0], np.cumsum(tb)])
    for core in range(NCORES):
        for blk in range(nblk):
            s, e = starts[core * nblk + blk], ends[core * nblk + blk]
            cnt = e - s
            t0 = tile_ofs[blk]
            flat = np.zeros(tb[blk] * P, dtype=np.int32)
            flat[:cnt] = src_s[s:e]
            dfl = np.full(tb[blk] * P, 255, dtype=np.uint8)
            dfl[:cnt] = dloc_s[s:e]
            idxsrc[core, t0:t0 + tb[blk], :] = flat.reshape(tb[blk], P)
            dlarr[core, t0:t0 + tb[blk], :] = dfl.reshape(tb[blk], P)
    # SBUF layout [nch, P, icc]: element [ch, p, t] = tile (ch*icc + t), lane p
    idxsrc = idxsrc.reshape(NCORES, nch, icc, P).transpose(0, 1, 3, 2).copy()
    dlarr = dlarr.reshape(NCORES, nch, icc, P).transpose(0, 1, 3, 2).copy()

    # self-row global indices per core,block (pad -> N, a zero row)
    selfidx = np.zeros((NCORES, nblk, P), dtype=np.int32)
    for core in range(NCORES):
        for blk in range(nblk):
            g0 = core * npc + blk * P
            nrow = P if blk < nblk - 1 else lastb
            ids = np.arange(g0, g0 + P, dtype=np.int32)
            ids[nrow:] = N
            selfidx[core, blk] = ids

    degB = np.ones((NCORES, nblk, P), dtype=np.float32)
    for core in range(NCORES):
        degB[core].reshape(-1)[:npc] = deg[core * npc:(core + 1) * npc]

    degA = np.ones(ncols * P, dtype=np.float32)
    degA[:N] = deg.astype(np.float32)
    degA = degA.reshape(ncols, P).T.copy()  # [128, ncols], p-major

    nxt = nrows // P
    xT = np.zeros((2, P, nrows), dtype=np.float32)
    xf = np.asarray(x, dtype=np.float32)
    xT[0, :, :N] = xf[:, :P].T
    xT[1, :, :N] = xf[:, P:].T
    # pre-tiled: [2, nxt, P(part), P] so each (k, j) tile is one contiguous 64KB
    xT = np.ascontiguousarray(
        xT.reshape(2, P, nxt, P).transpose(0, 2, 1, 3))

    W1c = np.ascontiguousarray(np.asarray(W1, dtype=np.float32).reshape(2, P, HID))
    Sd = np.zeros((OUT, 2 * HEADS), dtype=np.float32)
    for h in range(HEADS):
        Sd[h * C_OUT:(h + 1) * C_OUT, h] = np.asarray(att_src[h], np.float32)
        Sd[h * C_OUT:(h + 1) * C_OUT, HEADS + h] = np.asarray(att_dst[h], np.float32)

    meta = dict(tb=[int(t) for t in tb], TT=TT, nch=nch, icc=icc)
    per_core = []
    for core in range(NCORES):
        per_core.append({
            "xT": xT,
            "W1c": W1c,
            "W2": np.asarray(W2, np.float32),
            "Sd1": Sd[:P],
            "Sd2": Sd[P:],
            "b1": np.asarray(b1, np.float32)[None, :],
            "b2": np.asarray(b2, np.float32)[None, :],
            "degA": degA,
            "degB": degB[core],
            "idxsrc": idxsrc[core],
            "dlarr": dlarr[core],
            "selfidx": selfidx[core],
        })
    return per_core, meta


def _build(meta, repeat=1, phases="ABGC", timing_mode=False, ablate=()):
    npc, nblk, lastb, ncols, nrows = _sizes()
    tb, TT, nch, icc = meta["tb"], meta["TT"], meta["nch"], meta["icc"]

    nc = bacc.Bacc(None, target_bir_lowering=False, num_swdge_queues=4)
    _swq = [0]
    def _rr(inst):
        q = _swq[0] % 4
        _swq[0] += 1
        if q:
            inst.ins.queue = f"qPoolDynamic{q}"
        return inst
    big = dict(kind="ExternalInput") if not timing_mode else {}
    xT_t = nc.dram_tensor("xT", [2, nrows // P, P, P], F32, **big)
    W1_t = nc.dram_tensor("W1c", [2, P, HID], F32, **big)
    W2_t = nc.dram_tensor("W2", [P, OUT], F32, **big)
    Sd1_t = nc.dram_tensor("Sd1", [P, 2 * HEADS], F32, **big)
    Sd2_t = nc.dram_tensor("Sd2", [OUT - P, 2 * HEADS], F32, **big)
    b1_t = nc.dram_tensor("b1", [1, HID], F32, **big)
    b2_t = nc.dram_tensor("b2", [1, OUT], F32, **big)
    degA_t = nc.dram_tensor("degA", [P, ncols], F32, kind="ExternalInput")
    degB_t = nc.dram_tensor("degB", [nblk, P], F32, kind="ExternalInput")
    idx_t = nc.dram_tensor("idxsrc", [nch, P, icc], I32, kind="ExternalInput")
    dl_t = nc.dram_tensor("dlarr", [nch, P, icc], U8, kind="ExternalInput")
    self_t = nc.dram_tensor("selfidx", [nblk, P], I32, kind="ExternalInput")
    out_t = nc.dram_tensor("out", [npc, OUT], F32, kind="ExternalOutput")

    tile_ofs = [0]
    for t in tb:
        tile_ofs.append(tile_ofs[-1] + t)
    blk_of_tile = []
    for blk, t in enumerate(tb):
        blk_of_tile += [blk] * t

    with tile.TileContext(nc) as tc:
        with (
            tc.tile_pool(name="persist", bufs=1) as pp,
            tc.tile_pool(name="dram", bufs=1, space="DRAM") as dram,
        ):
            table1 = dram.tile([nrows, HID], F32)
            agb = dram.tile([nblk * P, T2W], F32)
            table2 = nc.dram_tensor("table2i", [nrows, T2W], F32,
                                    addr_space="Shared")

            ident = pp.tile([P, P], F32)
            make_identity(nc, ident[:])
            iota_i = pp.tile([P, P], I32)
            nc.gpsimd.iota(iota_i[:], pattern=[[1, P]], channel_multiplier=0)
            iota_row = pp.tile([P, P], F32)
            nc.vector.tensor_copy(iota_row[:], iota_i[:])
            iotac_i = pp.tile([P, 1], I32)
            nc.gpsimd.iota(iotac_i[:], pattern=[[0, 1]], channel_multiplier=1)
            iota_col = pp.tile([P, 1], F32)
            nc.vector.tensor_copy(iota_col[:], iotac_i[:])

            ones_row = pp.tile([1, P], F32)
            nc.vector.memset(ones_row[:], 1.0)
            b1r = pp.tile([1, HID], F32)
            nc.sync.dma_start(out=b1r[:], in_=b1_t[:, :])
            b2r = pp.tile([1, OUT], F32)
            nc.sync.dma_start(out=b2r[:], in_=b2_t[:, :])
            Wcat = pp.tile([P, OUT + 2 * HEADS], F32)
            nc.sync.dma_start(out=Wcat[:, :OUT], in_=W2_t[:, :])
            sd1 = pp.tile([P, 2 * HEADS], F32)
            nc.sync.dma_start(out=sd1[:], in_=Sd1_t[:, :])
            sd2 = pp.tile([OUT - P, 2 * HEADS], F32)
            nc.sync.dma_start(out=sd2[:], in_=Sd2_t[:, :])
            b1b = pp.tile([P, HID], F32)
            b2b = pp.tile([P, OUT], F32)
            w2t1 = pp.tile([P, P], F32)
            w2t2 = pp.tile([OUT - P, P], F32)
            with tc.tile_pool(name="pset", bufs=1, space="PSUM") as pset:
                b1b_ps = pset.tile([P, HID], F32, space="PSUM")
                nc.tensor.matmul(b1b_ps[:], ones_row[:], b1r[:], start=True, stop=True)
                nc.vector.tensor_copy(b1b[:], b1b_ps[:])
                b2b_ps = pset.tile([P, OUT], F32, space="PSUM")
                nc.tensor.matmul(b2b_ps[:], ones_row[:], b2r[:], start=True, stop=True)
                nc.vector.tensor_copy(b2b[:], b2b_ps[:])
                w2t1_ps = pset.tile([P, P], F32, space="PSUM")
                nc.tensor.transpose(w2t1_ps[:], Wcat[:, :P], ident[:])
                nc.vector.tensor_copy(w2t1[:], w2t1_ps[:])
                w2t2_ps = pset.tile([OUT - P, P], F32, space="PSUM")
                nc.tensor.transpose(w2t2_ps[:], Wcat[:, P:OUT], ident[:])
                nc.vector.tensor_copy(w2t2[:], w2t2_ps[:])
                sp_ps = pset.tile([P, 2 * HEADS], F32, space="PSUM")
                nc.tensor.matmul(sp_ps[:], w2t1[:], sd1[:], start=True, stop=False)
                nc.tensor.matmul(sp_ps[:], w2t2[:], sd2[:], start=False, stop=True)
                nc.vector.tensor_copy(Wcat[:, OUT:], sp_ps[:])

            W1sb = pp.tile([P, 2 * HID], F32)
            nc.sync.dma_start(out=W1sb[:, :HID], in_=W1_t[0])
            nc.sync.dma_start(out=W1sb[:, HID:], in_=W1_t[1])

            degA_sb = pp.tile([P, ncols], F32)
            nc.sync.dma_start(out=degA_sb[:], in_=degA_t[:, :])
            sqA = pp.tile([P, ncols], F32)
            nc.scalar.activation(sqA[:], degA_sb[:], AF.Sqrt)
            dinvA = pp.tile([P, ncols], F32)
            nc.vector.reciprocal(dinvA[:], sqA[:])

            # zero table2 rows [N, nrows) (pad gathers land here)
            zrow = pp.tile([P, T2W], F32)
            nc.vector.memset(zrow[:], 0.0)
            if nblk * P > npc:
                nc.sync.dma_start(out=agb[:][npc:nblk * P, :],
                                  in_=zrow[:nblk * P - npc, :])
            z0 = N
            while z0 < nrows:
                zn = min(P, nrows - z0)
                nc.sync.dma_start(out=table2[z0:z0 + zn, :], in_=zrow[:zn, :])
                z0 += zn

            for _ in range(repeat):
                # ---------------- Phase A: full table1 on every core ----------
                with (
                    tc.tile_pool(name="pa_x", bufs=4) as pa_x,
                    tc.tile_pool(name="pa_o", bufs=3) as pa_o,
                    tc.tile_pool(name="pa_ps", bufs=2, space="PSUM") as pa_ps,
                ):
                    for j in range(ncols + 1):  # last col covers zero pad rows
                        xa = pa_x.tile([P, 2 * P], F32, tag="xa")
                        nc.sync.dma_start(out=xa[:, :P], in_=xT_t[0, j])
                        nc.sync.dma_start(out=xa[:, P:], in_=xT_t[1, j])
                        psA = pa_ps.tile([P, HID], F32, space="PSUM", tag="psA")
                        nc.tensor.matmul(psA[:], xa[:, :P], W1sb[:, :HID],
                                         start=True, stop=False)
                        nc.tensor.matmul(psA[:], xa[:, P:], W1sb[:, HID:],
                                         start=False, stop=True)
                        xwp = pa_o.tile([P, HID], F32, tag="xwp")
                        if j < ncols:
                            nc.vector.tensor_scalar_mul(xwp[:], psA[:],
                                                        dinvA[:, j:j + 1])
                        else:
                            nc.vector.tensor_scalar_mul(xwp[:], psA[:], 0.0)
                        nc.sync.dma_start(out=table1[:][j * P:(j + 1) * P, :], in_=xwp[:])

                # ---------------- Phase B: GCN edge aggregation ---------------
                if "B" not in phases:
                    continue
                with (
                    tc.tile_pool(name="pb_idx", bufs=2) as pb_idx,
                    tc.tile_pool(name="pb_g", bufs=8) as pb_g,
                    tc.tile_pool(name="pb_se", bufs=4) as pb_se,
                    tc.tile_pool(name="pb_ps", bufs=2, space="PSUM") as pb_ps,
                    tc.tile_pool(name="pb_ep", bufs=2) as pb_ep,
                    tc.tile_pool(name="pb_ps2", bufs=2, space="PSUM") as pb_ps2,
                ):
                    idxc = dfc = ps1 = None
                    for t in range(TT):
                        ch, tci = divmod(t, icc)
                        if tci == 0:
                            idxc = pb_idx.tile([P, icc], I32, tag="idxc")
                            nc.sync.dma_start(out=idxc[:], in_=idx_t[ch])
                            dlc = pb_idx.tile([P, icc], U8, tag="dlc")
                            nc.sync.dma_start(out=dlc[:], in_=dl_t[ch])
                            dfc = pb_idx.tile([P, icc], F32, tag="dfc")
                            nc.vector.tensor_copy(dfc[:], dlc[:])
                        blk = blk_of_tile[t]
                        first = t == tile_ofs[blk]
                        last = t == tile_ofs[blk + 1] - 1
                        if first:
                            ps1 = pb_ps.tile([P, HID], F32, space="PSUM", tag="ps1")
                        g1 = pb_g.tile([P, HID], F32, tag="g1")
                        _rr(nc.gpsimd.indirect_dma_start(
                            out=g1[:], out_offset=None, in_=table1[:][:, :],
                            in_offset=bass.IndirectOffsetOnAxis(
                                ap=idxc[:, tci:tci + 1], axis=0)))
                        se = pb_se.tile([P, P], F32, tag="se")
                        nc.vector.tensor_tensor(
                            out=se[:], in0=dfc[:, tci:tci + 1].to_broadcast([P, P]),
                            in1=iota_row[:], op=ALU.is_equal)
                        nc.tensor.matmul(ps1[:], se[:], g1[:], start=first, stop=last)
                        if last:
                            sidx = pb_ep.tile([P, 1], I32, tag="sidx")
                            nc.sync.dma_start(out=sidx[:], in_=self_t[blk, :, None])
                            xwd = pb_ep.tile([P, HID], F32, tag="xwd")
                            nc.gpsimd.indirect_dma_start(
                                out=xwd[:], out_offset=None, in_=table1[:][:, :],
                                in_offset=bass.IndirectOffsetOnAxis(ap=sidx[:, :1], axis=0))
                            degb = pb_ep.tile([P, 1], F32, tag="degb")
                            nc.sync.dma_start(out=degb[:], in_=degB_t[blk, :, None])
                            sqb = pb_ep.tile([P, 1], F32, tag="sqb")
                            nc.scalar.activation(sqb[:], degb[:], AF.Sqrt)
                            dinvb = pb_ep.tile([P, 1], F32, tag="dinvb")
                            nc.vector.reciprocal(dinvb[:], sqb[:])
                            # h = relu(dinv*(ps1 + xwd) + b1); xwd already has
                            # one dinv factor folded in (it comes from table1)
                            tmp = pb_ep.tile([P, HID], F32, tag="tmp")
                            nc.vector.tensor_add(out=tmp[:], in0=xwd[:], in1=ps1[:])
                            nc.scalar.activation(tmp[:], tmp[:], AF.Copy,
                                                 scale=dinvb[:, :1])
                            nc.vector.tensor_add(out=tmp[:], in0=tmp[:], in1=b1b[:])
                            h = pb_ep.tile([P, HID], F32, tag="h")
                            nc.vector.tensor_scalar_max(h[:], tmp[:], 0.0)
                            hT_ps = pb_ps2.tile([P, P], F32, space="PSUM", tag="hT")
                            nc.tensor.transpose(hT_ps[:], h[:], ident[:])
                            hTs = pb_ep.tile([P, P], F32, tag="hTs")
                            nc.vector.tensor_copy(hTs[:], hT_ps[:])
                            ps2 = pb_ps2.tile([P, OUT + 2 * HEADS], F32,
                                              space="PSUM", tag="ps2")
                            nc.tensor.matmul(ps2[:], hTs[:], Wcat[:],
                                             start=True, stop=True)
                            stage = pb_ep.tile([P, T2W], F32, tag="stage")
                            nc.vector.tensor_copy(stage[:, :OUT + 2 * HEADS], ps2[:])
                            nrow = P if blk < nblk - 1 else lastb
                            nc.sync.dma_start(
                                out=agb[:][blk * P:blk * P + nrow, :OUT + 2 * HEADS],
                                in_=stage[:nrow, :OUT + 2 * HEADS])

                # ---------------- AllGather table2 ----------------------------
                if "G" not in phases:
                    pass
                else:
                    nc.gpsimd.collective_compute(
                        "AllGather", ALU.bypass,
                        replica_groups=[list(range(NCORES))],
                        ins=[agb[:][:npc, :].opt()],
                        outs=[table2[:NCORES * npc, :].opt()],
                    )

                # ---------------- Phase C: GAT --------------------------------
                if "C" not in phases:
                    continue
                maxtb = max(tb)
                with (
                    tc.tile_pool(name="pc_idx", bufs=2) as pc_idx,
                    tc.tile_pool(name="pc_g", bufs=maxtb + 4) as pc_g,
                    tc.tile_pool(name="pc_se", bufs=maxtb + 4) as pc_se,
                    tc.tile_pool(name="pc_gs", bufs=4) as pc_gs,
                    tc.tile_pool(name="pc_w", bufs=2) as pc_w,
                    tc.tile_pool(name="pc_ld", bufs=2) as pc_ld,
                    tc.tile_pool(name="pc_ps", bufs=2, space="PSUM") as pc_ps,
                    tc.tile_pool(name="pc_pst", bufs=2, space="PSUM") as pc_pst,
                    tc.tile_pool(name="pc_psa", bufs=2, space="PSUM") as pc_psa,
                    tc.tile_pool(name="pc_ep", bufs=2) as pc_ep,
                ):
                    idxc = dfc = None
                    for blk in range(nblk):
                        nt = tb[blk]
                        t0 = tile_ofs[blk]
                        psc = pc_ps.tile([P, NW], F32, space="PSUM", tag="psc")
                        ld = pc_ld.tile([P, T2W], F32, tag="ld")
                        nc.sync.dma_start(out=ld[:],
                                          in_=agb[:][blk * P:(blk + 1) * P, :])
                        wlb = pc_w.tile([P, HEADS * (maxtb + 1)], F32, tag="wlb")
                        g2s, ses = [], []
                        for tr in range(nt):
                            t = t0 + tr
                            ch, tci = divmod(t, icc)
                            if tci == 0:
                                idxc = pc_idx.tile([P, icc], I32, tag="idxc")
                                nc.sync.dma_start(out=idxc[:], in_=idx_t[ch])
                                dlc = pc_idx.tile([P, icc], U8, tag="dlc")
                                nc.sync.dma_start(out=dlc[:], in_=dl_t[ch])
                                dfc = pc_idx.tile([P, icc], F32, tag="dfc")
                                nc.vector.tensor_copy(dfc[:], dlc[:])
                            g2 = pc_g.tile([P, T2W], F32, tag="g2")
                            if "gather" not in ablate:
                                _rr(nc.gpsimd.indirect_dma_start(
                                    out=g2[:], out_offset=None, in_=table2[:, :],
                                    in_offset=bass.IndirectOffsetOnAxis(
                                        ap=idxc[:, tci:tci + 1], axis=0)))
                            if "onehot" not in ablate:
                                se = pc_se.tile([P, P], F32, tag="se")
                                nc.vector.tensor_tensor(
                                    out=se[:],
                                    in0=dfc[:, tci:tci + 1].to_broadcast([P, P]),
                                    in1=iota_row[:], op=ALU.is_equal)
                            else:
                                se = iota_row
                            if "adE" not in ablate:
                                dT_ps = pc_pst.tile([P, P], F32, space="PSUM", tag="dT")
                                nc.tensor.transpose(
                                    dT_ps[:], dfc[:, tci:tci + 1].to_broadcast([P, P]),
                                    ident[:])
                                st = pc_se.tile([P, P], F32, tag="st")
                                nc.vector.tensor_tensor(
                                    out=st[:], in0=dT_ps[:],
                                    in1=iota_col[:].to_broadcast([P, P]),
                                    op=ALU.is_equal)
                                ade_ps = pc_psa.tile([P, HEADS], F32, space="PSUM",
                                                     tag="ade")
                                nc.tensor.matmul(ade_ps[:], st[:],
                                                 ld[:, OUT + HEADS:OUT + 2 * HEADS],
                                                 start=True, stop=True)
                                nc.vector.tensor_add(
                                    out=wlb[:, HEADS * tr:HEADS * (tr + 1)],
                                    in0=g2[:, OUT:OUT + HEADS], in1=ade_ps[:])
                            else:
                                nc.vector.tensor_add(
                                    out=wlb[:, HEADS * tr:HEADS * (tr + 1)],
                                    in0=g2[:, OUT:OUT + HEADS],
                                    in1=g2[:, OUT:OUT + HEADS])
                            g2s.append(g2)
                            ses.append(se)
                        # self logit into the last slot
                        nc.vector.tensor_add(
                            out=wlb[:, HEADS * nt:HEADS * (nt + 1)],
                            in0=ld[:, OUT:OUT + HEADS],
                            in1=ld[:, OUT + HEADS:OUT + 2 * HEADS])
                        # block-wide lrelu + exp
                        nw_l = HEADS * (nt + 1)
                        wlsb = pc_w.tile([P, HEADS * (maxtb + 1)], F32, tag="wlsb")
                        nc.vector.tensor_scalar_mul(wlsb[:, :nw_l], wlb[:, :nw_l],
                                                    NEG_SLOPE)
                        nc.vector.tensor_tensor(out=wlsb[:, :nw_l], in0=wlb[:, :nw_l],
                                                in1=wlsb[:, :nw_l], op=ALU.max)
                        web = pc_w.tile([P, HEADS * (maxtb + 1)], F32, tag="web")
                        if "exp" not in ablate:
                            nc.scalar.activation(web[:, :nw_l], wlsb[:, :nw_l], AF.Exp)
                        else:
                            nc.vector.tensor_copy(web[:, :nw_l], wlsb[:, :nw_l])
                        # pass 2: scale + aggregate
                        for tr in range(nt):
                            gs = pc_gs.tile([P, NW], F32, tag="gs")
                            wsl = web[:, HEADS * tr:HEADS * (tr + 1)]
                            if "gscale" not in ablate:
                                nc.vector.tensor_tensor(
                                    out=gs[:, :OUT].rearrange("p (h c) -> p h c",
                                                              h=HEADS),
                                    in0=g2s[tr][:, :OUT].rearrange(
                                        "p (h c) -> p h c", h=HEADS),
                                    in1=_bcast_heads(wsl), op=ALU.mult)
                            nc.vector.tensor_copy(gs[:, OUT:NW], wsl)
                            if "mm" not in ablate or tr == nt - 1:
                                nc.tensor.matmul(psc[:], ses[tr][:], gs[:, :NW],
                                                 start=(tr == 0 or "mm" in ablate),
                                                 stop=tr == nt - 1)
                        # epilogue
                        wse = web[:, HEADS * nt:HEADS * (nt + 1)]
                        num = pc_ep.tile([P, OUT], F32, tag="num")
                        nc.vector.tensor_tensor(
                            out=num[:].rearrange("p (h c) -> p h c", h=HEADS),
                            in0=ld[:, :OUT].rearrange("p (h c) -> p h c", h=HEADS),
                            in1=_bcast_heads(wse), op=ALU.mult)
                        nc.vector.tensor_add(out=num[:], in0=num[:], in1=psc[:, :OUT])
                        den = pc_ep.tile([P, HEADS], F32, tag="den")
                        nc.vector.tensor_add(out=den[:], in0=wse, in1=psc[:, OUT:NW])
                        rden = pc_ep.tile([P, HEADS], F32, tag="rden")
                        nc.vector.reciprocal(rden[:], den[:])
                        o1 = pc_ep.tile([P, OUT], F32, tag="o1")
                        nc.vector.tensor_tensor(
                            out=o1[:].rearrange("p (h c) -> p h c", h=HEADS),
                            in0=num[:].rearrange("p (h c) -> p h c", h=HEADS),
                            in1=_bcast_heads(rden[:]), op=ALU.mult)
                        nc.vector.tensor_add(out=o1[:], in0=o1[:], in1=b2b[:])
                        o2 = pc_ep.tile([P, OUT], F32, tag="o2")
                        nc.vector.tensor_scalar_max(o2[:], o1[:], 0.0)
                        nrow = P if blk < nblk - 1 else lastb
                        nc.sync.dma_start(
                            out=out_t[blk * P:blk * P + nrow, :],
                            in_=o2[:nrow, :])
    return nc


def kernel(**inputs):
    per_core, meta = _host_prep(**inputs)
    nc = _build(meta)
    nc.compile()
    res = run_bass_kernel_spmd(nc, per_core, list(range(NCORES)))
    out = np.concatenate([res.results[c]["out"] for c in range(NCORES)], axis=0)
    return out.astype(np.float32)



# revision 6
# speedup vs baseline: 3.1706x; 3.1706x over previous
"""EnhancedGCN (GCNConv + GATConv) Trainium2 Bass kernel, 8-core SPMD.

Strategy (dst-node partitioned, per the sharding hint):
  - Host: sort edges by destination block, pad to uniform per-block tile
    counts across cores (one SPMD program), compute degrees (the CSR
    histogram) as part of partitioning. Self-loop terms are handled
    analytically per block on device (no gather needed for them).
  - Phase A (replicated): every core computes the full scaled table
    xw' = deg^-1/2 * (x @ W1)  ->  table1 [N, 128] in local HBM.
  - Phase B (dst-partitioned): per 128-dst block, per 128-edge tile:
    indirect-DMA gather of xw'[src] rows, one-hot selection matrix from
    dst-local ids (DVE is_equal vs iota), PE matmul accumulation into
    PSUM. Block epilogue: add self-loop term, scale by deg^-1/2[dst],
    +b1, ReLU -> h; transpose h; one matmul with [W2 | S'src | S'dst]
    gives [h2 | a_s | a_d] rows -> staged for exchange.
  - AllGather of the per-core [h2 | a_s | a_d] slices -> table2 [N, 200].
  - Phase C: per edge tile: gather table2[src] rows ([h2|a_s]), replicate
    a_d[dst] to edges via PE transpose + one-hot matmul, logits =
    a_s+a_d -> LeakyReLU -> Exp (no max-subtraction; logits are O(10)),
    scale h2 rows by per-head weights, matmul with one-hot accumulates
    [weighted h2 | weight] per dst; epilogue adds the self-loop term,
    divides by the summed weights, +b2, ReLU -> output rows.
"""
import math
import numpy as np

import concourse.bass as bass
from concourse import bacc
import concourse.mybir as mybir
import concourse.tile as tile
from concourse.masks import make_identity
from concourse.bass_utils import run_bass_kernel_spmd

P = 128
F32 = mybir.dt.float32
I32 = mybir.dt.int32
U8 = mybir.dt.uint8
AF = mybir.ActivationFunctionType
ALU = mybir.AluOpType

# Problem constants (tests may override N before calling kernel())
N = 50000
IN_CH = 256
HID = 128
HEADS = 3
C_OUT = 64
OUT = HEADS * C_OUT  # 192
NEG_SLOPE = 0.2
NCORES = 8
T2W = 200            # table2 row width (192 h2 + 3 a_s + 3 a_d + 2 pad)
IC = 512             # idx chunk columns
NW = OUT + HEADS     # 195: [G' | w] matmul width


def _sizes():
    npc = N // NCORES
    nblk = math.ceil(npc / P)
    lastb = npc - (nblk - 1) * P
    ncols = math.ceil(N / P)
    nrows = ncols * P + P  # extra zero rows for pad gathers
    return npc, nblk, lastb, ncols, nrows


def _bcast_heads(ap):
    """[P, HEADS] AP -> [P, HEADS, C_OUT] zero-step broadcast AP."""
    return bass.AP(ap.tensor, ap.offset, [ap.ap[0], [1, HEADS], [0, C_OUT]])


def _host_prep(x, edge_index, W1, b1, W2, att_src, att_dst, b2):
    npc, nblk, lastb, ncols, nrows = _sizes()
    src = np.asarray(edge_index[0], dtype=np.int64)
    dst = np.asarray(edge_index[1], dtype=np.int64)
    deg = np.bincount(dst, minlength=N).astype(np.float64) + 1.0  # + self loop

    c = dst // npc
    r = dst - c * npc
    b = r // P
    cb = (c * nblk + b).astype(np.int64)
    dloc = (r - b * P).astype(np.int64)

    nbuckets = NCORES * nblk
    counts = np.bincount(cb, minlength=nbuckets).reshape(NCORES, nblk)
    tb = np.maximum(1, np.ceil(counts.max(axis=0) / P).astype(np.int64))  # [nblk]
    TT = int(tb.sum())
    nch = math.ceil(TT / IC)
    icc = min(IC, TT)

    order = np.argsort(cb, kind="stable")
    src_s = src[order]
    dloc_s = dloc[order]
    cb_s = cb[order]
    starts = np.searchsorted(cb_s, np.arange(nbuckets))
    ends = np.searchsorted(cb_s, np.arange(nbuckets) + 1)

    idxsrc = np.zeros((NCORES, nch * icc, P), dtype=np.int32)
    dlarr = np.full((NCORES, nch * icc, P), 255, dtype=np.uint8)
    tile_ofs = np.concatenate([[0], np.cumsum(tb)])
    for core in range(NCORES):
        for blk in range(nblk):
            s, e = starts[core * nblk + blk], ends[core * nblk + blk]
            cnt = e - s
            t0 = tile_ofs[blk]
            flat = np.zeros(tb[blk] * P, dtype=np.int32)
            flat[:cnt] = src_s[s:e]
            dfl = np.full(tb[blk] * P, 255, dtype=np.uint8)
            dfl[:cnt] = dloc_s[s:e]
            idxsrc[core, t0:t0 + tb[blk], :] = flat.reshape(tb[blk], P)
            dlarr[core, t0:t0 + tb[blk], :] = dfl.reshape(tb[blk], P)
    # SBUF layout [nch, P, icc]: element [ch, p, t] = tile (ch*icc + t), lane p
    idxsrc = idxsrc.reshape(NCORES, nch, icc, P).transpose(0, 1, 3, 2).copy()
    dlarr = dlarr.reshape(NCORES, nch, icc, P).transpose(0, 1, 3, 2).copy()

    # self-row global indices per core,block (pad -> N, a zero row)
    selfidx = np.zeros((NCORES, nblk, P), dtype=np.int32)
    for core in range(NCORES):
        for blk in range(nblk):
            g0 = core * npc + blk * P
            nrow = P if blk < nblk - 1 else lastb
            ids = np.arange(g0, g0 + P, dtype=np.int32)
            ids[nrow:] = N
            selfidx[core, blk] = ids

    degB = np.ones((NCORES, nblk, P), dtype=np.float32)
    for core in range(NCORES):
        degB[core].reshape(-1)[:npc] = deg[core * npc:(core + 1) * npc]

    degA = np.ones(ncols * P, dtype=np.float32)
    degA[:N] = deg.astype(np.float32)
    degA = degA.reshape(ncols, P).T.copy()  # [128, ncols], p-major

    nxt = nrows // P
    xT = np.zeros((2, P, nrows), dtype=np.float32)
    xf = np.asarray(x, dtype=np.float32)
    xT[0, :, :N] = xf[:, :P].T
    xT[1, :, :N] = xf[:, P:].T
    # pre-tiled: [2, nxt, P(part), P] so each (k, j) tile is one contiguous 64KB
    xT = np.ascontiguousarray(
        xT.reshape(2, P, nxt, P).transpose(0, 2, 1, 3))

    W1c = np.ascontiguousarray(np.asarray(W1, dtype=np.float32).reshape(2, P, HID))
    Sd = np.zeros((OUT, 2 * HEADS), dtype=np.float32)
    for h in range(HEADS):
        Sd[h * C_OUT:(h + 1) * C_OUT, h] = np.asarray(att_src[h], np.float32)
        Sd[h * C_OUT:(h + 1) * C_OUT, HEADS + h] = np.asarray(att_dst[h], np.float32)

    meta = dict(tb=[int(t) for t in tb], TT=TT, nch=nch, icc=icc)
    per_core = []
    for core in range(NCORES):
        per_core.append({
            "xT": xT,
            "W1c": W1c,
            "W2": np.asarray(W2, np.float32),
            "Sd1": Sd[:P],
            "Sd2": Sd[P:],
            "b1": np.asarray(b1, np.float32)[None, :],
            "b2": np.asarray(b2, np.float32)[None, :],
            "degA": degA,
            "degB": degB[core],
            "idxsrc": idxsrc[core],
            "dlarr": dlarr[core],
            "selfidx": selfidx[core],
        })
    return per_core, meta


def _build(meta, repeat=1, phases="ABGC", timing_mode=False, ablate=()):
    npc, nblk, lastb, ncols, nrows = _sizes()
    tb, TT, nch, icc = meta["tb"], meta["TT"], meta["nch"], meta["icc"]

    nc = bacc.Bacc(None, target_bir_lowering=False, num_swdge_queues=4)
    _swq = [0]
    def _rr(inst):
        q = _swq[0] % 4
        _swq[0] += 1
        if q:
            inst.ins.queue = f"qPoolDynamic{q}"
        return inst
    big = dict(kind="ExternalInput") if not timing_mode else {}
    xT_t = nc.dram_tensor("xT", [2, nrows // P, P, P], F32, **big)
    W1_t = nc.dram_tensor("W1c", [2, P, HID], F32, **big)
    W2_t = nc.dram_tensor("W2", [P, OUT], F32, **big)
    Sd1_t = nc.dram_tensor("Sd1", [P, 2 * HEADS], F32, **big)
    Sd2_t = nc.dram_tensor("Sd2", [OUT - P, 2 * HEADS], F32, **big)
    b1_t = nc.dram_tensor("b1", [1, HID], F32, **big)
    b2_t = nc.dram_tensor("b2", [1, OUT], F32, **big)
    degA_t = nc.dram_tensor("degA", [P, ncols], F32, kind="ExternalInput")
    degB_t = nc.dram_tensor("degB", [nblk, P], F32, kind="ExternalInput")
    idx_t = nc.dram_tensor("idxsrc", [nch, P, icc], I32, kind="ExternalInput")
    dl_t = nc.dram_tensor("dlarr", [nch, P, icc], U8, kind="ExternalInput")
    self_t = nc.dram_tensor("selfidx", [nblk, P], I32, kind="ExternalInput")
    out_t = nc.dram_tensor("out", [npc, OUT], F32, kind="ExternalOutput")

    tile_ofs = [0]
    for t in tb:
        tile_ofs.append(tile_ofs[-1] + t)
    blk_of_tile = []
    for blk, t in enumerate(tb):
        blk_of_tile += [blk] * t

    with tile.TileContext(nc) as tc:
        with (
            tc.tile_pool(name="persist", bufs=1) as pp,
            tc.tile_pool(name="dram", bufs=1, space="DRAM") as dram,
        ):
            table1 = dram.tile([nrows, HID], F32)
            agb = dram.tile([nblk * P, T2W], F32)
            table2 = nc.dram_tensor("table2i", [nrows, T2W], F32,
                                    addr_space="Shared")

            ident = pp.tile([P, P], F32)
            make_identity(nc, ident[:])
            iota_i = pp.tile([P, P], I32)
            nc.gpsimd.iota(iota_i[:], pattern=[[1, P]], channel_multiplier=0)
            iota_row = pp.tile([P, P], F32)
            nc.vector.tensor_copy(iota_row[:], iota_i[:])
            iotac_i = pp.tile([P, 1], I32)
            nc.gpsimd.iota(iotac_i[:], pattern=[[0, 1]], channel_multiplier=1)
            iota_col = pp.tile([P, 1], F32)
            nc.vector.tensor_copy(iota_col[:], iotac_i[:])

            ones_row = pp.tile([1, P], F32)
            nc.vector.memset(ones_row[:], 1.0)
            b1r = pp.tile([1, HID], F32)
            nc.sync.dma_start(out=b1r[:], in_=b1_t[:, :])
            b2r = pp.tile([1, OUT], F32)
            nc.sync.dma_start(out=b2r[:], in_=b2_t[:, :])
            Wcat = pp.tile([P, OUT + 2 * HEADS], F32)
            nc.sync.dma_start(out=Wcat[:, :OUT], in_=W2_t[:, :])
            sd1 = pp.tile([P, 2 * HEADS], F32)
            nc.sync.dma_start(out=sd1[:], in_=Sd1_t[:, :])
            sd2 = pp.tile([OUT - P, 2 * HEADS], F32)
            nc.sync.dma_start(out=sd2[:], in_=Sd2_t[:, :])
            b1b = pp.tile([P, HID], F32)
            b2b = pp.tile([P, OUT], F32)
            w2t1 = pp.tile([P, P], F32)
            w2t2 = pp.tile([OUT - P, P], F32)
            with tc.tile_pool(name="pset", bufs=1, space="PSUM") as pset:
                b1b_ps = pset.tile([P, HID], F32, space="PSUM")
                nc.tensor.matmul(b1b_ps[:], ones_row[:], b1r[:], start=True, stop=True)
                nc.vector.tensor_copy(b1b[:], b1b_ps[:])
                b2b_ps = pset.tile([P, OUT], F32, space="PSUM")
                nc.tensor.matmul(b2b_ps[:], ones_row[:], b2r[:], start=True, stop=True)
                nc.vector.tensor_copy(b2b[:], b2b_ps[:])
                w2t1_ps = pset.tile([P, P], F32, space="PSUM")
                nc.tensor.transpose(w2t1_ps[:], Wcat[:, :P], ident[:])
                nc.vector.tensor_copy(w2t1[:], w2t1_ps[:])
                w2t2_ps = pset.tile([OUT - P, P], F32, space="PSUM")
                nc.tensor.transpose(w2t2_ps[:], Wcat[:, P:OUT], ident[:])
                nc.vector.tensor_copy(w2t2[:], w2t2_ps[:])
                sp_ps = pset.tile([P, 2 * HEADS], F32, space="PSUM")
                nc.tensor.matmul(sp_ps[:], w2t1[:], sd1[:], start=True, stop=False)
                nc.tensor.matmul(sp_ps[:], w2t2[:], sd2[:], start=False, stop=True)
                nc.vector.tensor_copy(Wcat[:, OUT:], sp_ps[:])

            W1sb = pp.tile([P, 2 * HID], F32)
            nc.sync.dma_start(out=W1sb[:, :HID], in_=W1_t[0])
            nc.sync.dma_start(out=W1sb[:, HID:], in_=W1_t[1])

            degA_sb = pp.tile([P, ncols], F32)
            nc.sync.dma_start(out=degA_sb[:], in_=degA_t[:, :])
            sqA = pp.tile([P, ncols], F32)
            nc.scalar.activation(sqA[:], degA_sb[:], AF.Sqrt)
            dinvA = pp.tile([P, ncols], F32)
            nc.vector.reciprocal(dinvA[:], sqA[:])

            # zero table2 rows [N, nrows) (pad gathers land here)
            zrow = pp.tile([P, T2W], F32)
            nc.vector.memset(zrow[:], 0.0)
            if nblk * P > npc:
                nc.sync.dma_start(out=agb[:][npc:nblk * P, :],
                                  in_=zrow[:nblk * P - npc, :])
            z0 = N
            while z0 < nrows:
                zn = min(P, nrows - z0)
                nc.sync.dma_start(out=table2[z0:z0 + zn, :], in_=zrow[:zn, :])
                z0 += zn

            for _ in range(repeat):
                # ---------------- Phase A: full table1 on every core ----------
                with (
                    tc.tile_pool(name="pa_x", bufs=4) as pa_x,
                    tc.tile_pool(name="pa_o", bufs=3) as pa_o,
                    tc.tile_pool(name="pa_ps", bufs=2, space="PSUM") as pa_ps,
                ):
                    for j in range(ncols + 1):  # last col covers zero pad rows
                        xa = pa_x.tile([P, 2 * P], F32, tag="xa")
                        nc.sync.dma_start(out=xa[:, :P], in_=xT_t[0, j])
                        nc.sync.dma_start(out=xa[:, P:], in_=xT_t[1, j])
                        psA = pa_ps.tile([P, HID], F32, space="PSUM", tag="psA")
                        nc.tensor.matmul(psA[:], xa[:, :P], W1sb[:, :HID],
                                         start=True, stop=False)
                        nc.tensor.matmul(psA[:], xa[:, P:], W1sb[:, HID:],
                                         start=False, stop=True)
                        xwp = pa_o.tile([P, HID], F32, tag="xwp")
                        if j < ncols:
                            nc.vector.tensor_scalar_mul(xwp[:], psA[:],
                                                        dinvA[:, j:j + 1])
                        else:
                            nc.vector.tensor_scalar_mul(xwp[:], psA[:], 0.0)
                        nc.sync.dma_start(out=table1[:][j * P:(j + 1) * P, :], in_=xwp[:])

                # ---------------- Phase B: GCN edge aggregation ---------------
                if "B" not in phases:
                    continue
                with (
                    tc.tile_pool(name="pb_idx", bufs=2) as pb_idx,
                    tc.tile_pool(name="pb_g", bufs=8) as pb_g,
                    tc.tile_pool(name="pb_se", bufs=4) as pb_se,
                    tc.tile_pool(name="pb_ps", bufs=2, space="PSUM") as pb_ps,
                    tc.tile_pool(name="pb_ep", bufs=2) as pb_ep,
                    tc.tile_pool(name="pb_ps2", bufs=2, space="PSUM") as pb_ps2,
                ):
                    idxc = dfc = ps1 = None
                    for t in range(TT):
                        ch, tci = divmod(t, icc)
                        if tci == 0:
                            idxc = pb_idx.tile([P, icc], I32, tag="idxc")
                            nc.sync.dma_start(out=idxc[:], in_=idx_t[ch])
                            dlc = pb_idx.tile([P, icc], U8, tag="dlc")
                            nc.sync.dma_start(out=dlc[:], in_=dl_t[ch])
                            dfc = pb_idx.tile([P, icc], F32, tag="dfc")
                            nc.vector.tensor_copy(dfc[:], dlc[:])
                        blk = blk_of_tile[t]
                        first = t == tile_ofs[blk]
                        last = t == tile_ofs[blk + 1] - 1
                        if first:
                            ps1 = pb_ps.tile([P, HID], F32, space="PSUM", tag="ps1")
                        g1 = pb_g.tile([P, HID], F32, tag="g1")
                        _rr(nc.gpsimd.indirect_dma_start(
                            out=g1[:], out_offset=None, in_=table1[:][:, :],
                            in_offset=bass.IndirectOffsetOnAxis(
                                ap=idxc[:, tci:tci + 1], axis=0)))
                        se = pb_se.tile([P, P], F32, tag="se")
                        nc.vector.tensor_tensor(
                            out=se[:], in0=dfc[:, tci:tci + 1].to_broadcast([P, P]),
                            in1=iota_row[:], op=ALU.is_equal)
                        nc.tensor.matmul(ps1[:], se[:], g1[:], start=first, stop=last)
                        if last:
                            sidx = pb_ep.tile([P, 1], I32, tag="sidx")
                            nc.sync.dma_start(out=sidx[:], in_=self_t[blk, :, None])
                            xwd = pb_ep.tile([P, HID], F32, tag="xwd")
                            nc.gpsimd.indirect_dma_start(
                                out=xwd[:], out_offset=None, in_=table1[:][:, :],
                                in_offset=bass.IndirectOffsetOnAxis(ap=sidx[:, :1], axis=0))
                            degb = pb_ep.tile([P, 1], F32, tag="degb")
                            nc.sync.dma_start(out=degb[:], in_=degB_t[blk, :, None])
                            sqb = pb_ep.tile([P, 1], F32, tag="sqb")
                            nc.scalar.activation(sqb[:], degb[:], AF.Sqrt)
                            dinvb = pb_ep.tile([P, 1], F32, tag="dinvb")
                            nc.vector.reciprocal(dinvb[:], sqb[:])
                            # h = relu(dinv*(ps1 + xwd) + b1); xwd already has
                            # one dinv factor folded in (it comes from table1)
                            tmp = pb_ep.tile([P, HID], F32, tag="tmp")
                            nc.vector.tensor_add(out=tmp[:], in0=xwd[:], in1=ps1[:])
                            nc.scalar.activation(tmp[:], tmp[:], AF.Copy,
                                                 scale=dinvb[:, :1])
                            nc.vector.tensor_add(out=tmp[:], in0=tmp[:], in1=b1b[:])
                            h = pb_ep.tile([P, HID], F32, tag="h")
                            nc.vector.tensor_scalar_max(h[:], tmp[:], 0.0)
                            hT_ps = pb_ps2.tile([P, P], F32, space="PSUM", tag="hT")
                            nc.tensor.transpose(hT_ps[:], h[:], ident[:])
                            hTs = pb_ep.tile([P, P], F32, tag="hTs")
                            nc.vector.tensor_copy(hTs[:], hT_ps[:])
                            ps2 = pb_ps2.tile([P, OUT + 2 * HEADS], F32,
                                              space="PSUM", tag="ps2")
                            nc.tensor.matmul(ps2[:], hTs[:], Wcat[:],
                                             start=True, stop=True)
                            stage = pb_ep.tile([P, T2W], F32, tag="stage")
                            nc.vector.tensor_copy(stage[:, :OUT + 2 * HEADS], ps2[:])
                            nrow = P if blk < nblk - 1 else lastb
                            nc.sync.dma_start(
                                out=agb[:][blk * P:blk * P + nrow, :OUT + 2 * HEADS],
                                in_=stage[:nrow, :OUT + 2 * HEADS])

                # ---------------- AllGather table2 ----------------------------
                if "G" not in phases:
                    pass
                else:
                    nc.gpsimd.collective_compute(
                        "AllGather", ALU.bypass,
                        replica_groups=[list(range(NCORES))],
                        ins=[agb[:][:npc, :].opt()],
                        outs=[table2[:NCORES * npc, :].opt()],
                    )

                # ---------------- Phase C: GAT --------------------------------
                if "C" not in phases:
                    continue
                maxtb = max(tb)
                with (
                    tc.tile_pool(name="pc_idx", bufs=2) as pc_idx,
                    tc.tile_pool(name="pc_g", bufs=maxtb + 4) as pc_g,
                    tc.tile_pool(name="pc_se", bufs=maxtb + 4) as pc_se,
                    tc.tile_pool(name="pc_gs", bufs=4) as pc_gs,
                    tc.tile_pool(name="pc_w", bufs=2) as pc_w,
                    tc.tile_pool(name="pc_ld", bufs=2) as pc_ld,
                    tc.tile_pool(name="pc_ps", bufs=2, space="PSUM") as pc_ps,
                    tc.tile_pool(name="pc_pst", bufs=2, space="PSUM") as pc_pst,
                    tc.tile_pool(name="pc_psa", bufs=2, space="PSUM") as pc_psa,
                    tc.tile_pool(name="pc_ep", bufs=2) as pc_ep,
                ):
                    idxc = dfc = None
                    for blk in range(nblk):
                        nt = tb[blk]
                        t0 = tile_ofs[blk]
                        psc = pc_ps.tile([P, NW], F32, space="PSUM", tag="psc")
                        ld = pc_ld.tile([P, T2W], F32, tag="ld")
                        nc.sync.dma_start(out=ld[:],
                                          in_=agb[:][blk * P:(blk + 1) * P, :])
                        wlb = pc_w.tile([P, HEADS * (maxtb + 1)], F32, tag="wlb")
                        g2s, ses = [], []
                        for tr in range(nt):
                            t = t0 + tr
                            ch, tci = divmod(t, icc)
                            if tci == 0:
                                idxc = pc_idx.tile([P, icc], I32, tag="idxc")
                                nc.sync.dma_start(out=idxc[:], in_=idx_t[ch])
                                dlc = pc_idx.tile([P, icc], U8, tag="dlc")
                                nc.sync.dma_start(out=dlc[:], in_=dl_t[ch])
                                dfc = pc_idx.tile([P, icc], F32, tag="dfc")
                                nc.vector.tensor_copy(dfc[:], dlc[:])
                            g2 = pc_g.tile([P, T2W], F32, tag="g2")
                            if "gather" not in ablate:
                                _rr(nc.gpsimd.indirect_dma_start(
                                    out=g2[:], out_offset=None, in_=table2[:, :],
                                    in_offset=bass.IndirectOffsetOnAxis(
                                        ap=idxc[:, tci:tci + 1], axis=0)))
                            if "onehot" not in ablate:
                                se = pc_se.tile([P, P], F32, tag="se")
                                nc.vector.tensor_tensor(
                                    out=se[:],
                                    in0=dfc[:, tci:tci + 1].to_broadcast([P, P]),
                                    in1=iota_row[:], op=ALU.is_equal)
                            else:
                                se = iota_row
                            if "adE" not in ablate:
                                dT_ps = pc_pst.tile([P, P], F32, space="PSUM", tag="dT")
                                nc.tensor.transpose(
                                    dT_ps[:], dfc[:, tci:tci + 1].to_broadcast([P, P]),
                                    ident[:])
                                st = pc_se.tile([P, P], F32, tag="st")
                                nc.vector.tensor_tensor(
                                    out=st[:], in0=dT_ps[:],
                                    in1=iota_col[:].to_broadcast([P, P]),
                                    op=ALU.is_equal)
                                ade_ps = pc_psa.tile([P, HEADS], F32, space="PSUM",
                                                     tag="ade")
                                nc.tensor.matmul(ade_ps[:], st[:],
                                                 ld[:, OUT + HEADS:OUT + 2 * HEADS],
                                                 start=True, stop=True)
                                nc.vector.tensor_add(
                                    out=wlb[:, HEADS * tr:HEADS * (tr + 1)],
                                    in0=g2[:, OUT:OUT + HEADS], in1=ade_ps[:])
                            else:
                                nc.vector.tensor_add(
                                    out=wlb[:, HEADS * tr:HEADS * (tr + 1)],
                                    in0=g2[:, OUT:OUT + HEADS],
                                    in1=g2[:, OUT:OUT + HEADS])
                            g2s.append(g2)
                            ses.append(se)
                        # self logit into the last slot
                        nc.vector.tensor_add(
                            out=wlb[:, HEADS * nt:HEADS * (nt + 1)],
                            in0=ld[:, OUT:OUT + HEADS],
                            in1=ld[:, OUT + HEADS:OUT + 2 * HEADS])
                        # block-wide lrelu + exp
                        nw_l = HEADS * (nt + 1)
                        wlsb = pc_w.tile([P, HEADS * (maxtb + 1)], F32, tag="wlsb")
                        nc.vector.tensor_scalar_mul(wlsb[:, :nw_l], wlb[:, :nw_l],
                                                    NEG_SLOPE)
                        nc.vector.tensor_tensor(out=wlsb[:, :nw_l], in0=wlb[:, :nw_l],
                                                in1=wlsb[:, :nw_l], op=ALU.max)
                        web = pc_w.tile([P, HEADS * (maxtb + 1)], F32, tag="web")
                        if "exp" not in ablate:
                            nc.scalar.activation(web[:, :nw_l], wlsb[:, :nw_l], AF.Exp)
                        else:
                            nc.vector.tensor_copy(web[:, :nw_l], wlsb[:, :nw_l])
                        # pass 2: scale + aggregate
                        for tr in range(nt):
                            gs = pc_gs.tile([P, NW], F32, tag="gs")
                            wsl = web[:, HEADS * tr:HEADS * (tr + 1)]
                            if "gscale" not in ablate:
                                nc.vector.tensor_tensor(
                                    out=gs[:, :OUT].rearrange("p (h c) -> p h c",
                                                              h=HEADS),
                                    in0=g2s[tr][:, :OUT].rearrange(
                                        "p (h c) -> p h c", h=HEADS),
                                    in1=_bcast_heads(wsl), op=ALU.mult)
                            nc.vector.tensor_copy(gs[:, OUT:NW], wsl)
                            if "mm" not in ablate or tr == nt - 1:
                                nc.tensor.matmul(psc[:], ses[tr][:], gs[:, :NW],
                                                 start=(tr == 0 or "mm" in ablate),
                                                 stop=tr == nt - 1)
                        # epilogue
                        wse = web[:, HEADS * nt:HEADS * (nt + 1)]
                        num = pc_ep.tile([P, OUT], F32, tag="num")
                        nc.vector.tensor_tensor(
                            out=num[:].rearrange("p (h c) -> p h c", h=HEADS),
                            in0=ld[:, :OUT].rearrange("p (h c) -> p h c", h=HEADS),
                            in1=_bcast_heads(wse), op=ALU.mult)
                        nc.vector.tensor_add(out=num[:], in0=num[:], in1=psc[:, :OUT])
                        den = pc_ep.tile([P, HEADS], F32, tag="den")
                        nc.vector.tensor_add(out=den[:], in0=wse, in1=psc[:, OUT:NW])
                        rden = pc_ep.tile([P, HEADS], F32, tag="rden")
                        nc.vector.reciprocal(rden[:], den[:])
                        o1 = pc_ep.tile([P, OUT], F32, tag="o1")
                        nc.vector.tensor_tensor(
                            out=o1[:].rearrange("p (h c) -> p h c", h=HEADS),
                            in0=num[:].rearrange("p (h c) -> p h c", h=HEADS),
                            in1=_bcast_heads(rden[:]), op=ALU.mult)
                        nc.vector.tensor_add(out=o1[:], in0=o1[:], in1=b2b[:])
                        o2 = pc_ep.tile([P, OUT], F32, tag="o2")
                        nc.vector.tensor_scalar_max(o2[:], o1[:], 0.0)
                        nrow = P if blk < nblk - 1 else lastb
                        nc.sync.dma_start(
                            out=out_t[blk * P:blk * P + nrow, :],
                            in_=o2[:nrow, :])
    return nc


def kernel(**inputs):
    per_core, meta = _host_prep(**inputs)
    nc = _build(meta)
    nc.compile()
    res = run_bass_kernel_spmd(nc, per_core, list(range(NCORES)))
    out = np.concatenate([res.results[c]["out"] for c in range(NCORES)], axis=0)
    return out.astype(np.float32)

